# revision 49
# baseline (speedup 1.0000x reference)
"""KDE log-density kernel for Trainium2, SPMD across 8 NeuronCores.

Computes log_p[m] = logsumexp_n(-scale * ||X[m] - svs[n]||^2) - log(N)
                    + (D/2) * log(scale/pi)

Sharding: 4-way over X rows x 2-way over svs rows (core i handles X
quarter i%4 against svs half i//4).  Each core returns the raw partial
sum T[m] = sum_n exp(-scale*||x_m - s_n||^2) over its svs half; the
host unshards by summing the two halves per query row and applying
log(T) + C.  This halves per-core DMA versus replicating svs, which is
what the kernel is otherwise bound by.

Per-core algorithm (fp8 DoubleRow pipeline):
  - svs (2 chunks of 2048 rows = one n-group each): DMA f32 load ->
    DVE cast 2*svs to fp8e4 -> DMA store fp8 -> DMA-transpose the
    uint16-bitcast (adjacent-d pairs packed per 16-bit element) into
    per-group svsT8 [128, 2, 2*NG] fp8 where (partition p, K, lane i)
    maps to d = 2*(K*128+p)+i.  Squares of svsT8 (for the -s2 inject)
    alternate Pool/DVE per 512-n slice.  Identical packing on both
    matmul operands keeps the DoubleRow contraction consistent.
  - X: f32 load in halves; x2 row-sums for half 0 on the idle ACT head
    (Square+accum), half 1 on DVE; cast/store/transpose per half.
  - Main loop over 2 n-groups x 16 m-tiles: per 512-n chunk, 2
    DoubleRow matmuls (K=256 each) accumulate 2*x.s into a psum bank,
    then 2 DoubleRow matmuls with an all-(-0.25) stationary against
    sqT8 add -s2[n] into the same bank.  One wide ACT exp over 4 banks
    [128, 2048] in-place with bias=-scale*x2[m], scale=scale,
    accum_out -> partials.  T = sum of the 2 group partials.

DMA queues: loads with no dependencies on SP; dependent stores and
transposes issue from the ACT queue (issue-only on its sequencer).
"""

import sys

for _p in ("/opt/trn_rl_repo", "/opt/pypackages"):
    if _p not in sys.path:
        sys.path.insert(0, _p)

import numpy as np

M_FULL, N_FULL, D = 8192, 8192, 512
NCORES = 8
XSH, SSH = 4, 2           # X shards x svs shards
M_LOC = M_FULL // XSH     # 2048
N_LOC = N_FULL // SSH     # 4096
P = 128
MT = M_LOC // P           # 16 m-tiles per core
NCH = 512                 # psum bank of f32
NGRP = 2                  # n-groups (one wide psum tile each per m-tile)
NG = N_LOC // NGRP        # 2048 n per group
RCH = 2048                # svs rows per pipeline chunk (= one n-group)
JCH = RCH // P            # 16 row-tiles per chunk
XH = M_LOC // 2           # X half rows (1024)

_CACHE = {}


def _build_nc():
    import concourse.mybir as mybir
    import concourse.tile as tile
    from concourse import bacc

    f32 = mybir.dt.float32
    fp8 = mybir.dt.float8e4
    u16 = mybir.dt.uint16
    AF = mybir.ActivationFunctionType
    ALU = mybir.AluOpType

    DR = mybir.MatmulPerfMode.DoubleRow

    nc = bacc.Bacc(None, target_bir_lowering=False, debug=True)
    Xd = nc.declare_dram_parameter("X", [M_LOC, D], f32, isOutput=False)
    Sd = nc.declare_dram_parameter("svs", [N_LOC, D], f32, isOutput=False)
    scd = nc.declare_dram_parameter("scale", [1], f32, isOutput=False)
    outd = nc.declare_dram_parameter("out", [NGRP, M_LOC], f32, isOutput=True)

    def drpair(ap2d, n0, ncols):
        # fp8 [128, 2*cols] packed-pair slice -> DoubleRow [128, 2, ncols]
        return ap2d[:, 2 * n0:2 * (n0 + ncols)].rearrange(
            "p (n two) -> p two n", two=2
        )

    with tile.TileContext(nc) as tc:
        with (
            tc.tile_pool(name="const", bufs=1) as cp,
            tc.tile_pool(name="stage", bufs=2) as stp,
            tc.tile_pool(name="s8", bufs=2) as s8p,
            tc.tile_pool(name="small", bufs=4) as sp,
            tc.tile_pool(name="wpsum", bufs=2, space="PSUM") as pp,
            tc.tile_pool(name="dram", bufs=1, space="DRAM") as dp,
        ):
            # ---------- constants ----------
            scale_bc = cp.tile([P, 1], f32)
            nc.sync.dma_start(scale_bc[:], scd[None, :].to_broadcast((P, 1)))
            negscale = cp.tile([P, 1], f32)
            nc.scalar.mul(negscale[:], scale_bc[:], -1.0)
            neg_q = cp.tile([P, 2, P], fp8)
            nc.gpsimd.memset(neg_q[:], -0.25)

            # resident tensors
            xst = cp.tile([P, MT, D], f32)          # X f32, row t*128+p
            x8 = cp.tile([P, MT, D], fp8)
            xT8 = cp.tile([P, 2, M_LOC * 2], fp8)   # packed pairs
            xT8p = cp.tile([P, 2, 2, M_LOC], fp8)   # planar [p, K, lane, m]
            x2_all = cp.tile([P, MT], f32)
            x2sc = cp.tile([P, MT], f32)
            svsT8 = [cp.tile([P, 2, NG * 2], fp8, name=f"svsT8{g}")
                     for g in range(NGRP)]
            sqT8 = [cp.tile([P, 2, NG * 2], fp8, name=f"sqT8{g}")
                    for g in range(NGRP)]
            partials = cp.tile([P, MT, NGRP], f32)

            x8d = dp.tile([M_LOC, D], fp8)
            s8d = dp.tile([N_LOC, D], fp8)

            # ---------- pipeline stages ----------
            def x_load(h):
                nc.sync.dma_start(
                    xst[:, h * 8:(h + 1) * 8, :],
                    Xd[h * XH:(h + 1) * XH, :].rearrange(
                        "(t p) d -> p t d", p=P),
                )

            def x_cast(h):
                nc.vector.tensor_copy(
                    x8[:, h * 8:(h + 1) * 8, :], xst[:, h * 8:(h + 1) * 8, :]
                )

            def x_store_transp(h):
                nc.sync.dma_start(
                    x8d[h * XH:(h + 1) * XH, :].rearrange(
                        "(t p) d -> p t d", p=P),
                    x8[:, h * 8:(h + 1) * 8, :],
                )
                nc.sync.dma_start_transpose(
                    xT8.bitcast(u16)[:, :, h * XH:(h + 1) * XH],
                    x8d.bitcast(u16)[h * XH:(h + 1) * XH, :],
                )
                # de-interleave to planar for the dual-fp8 ldweights ISA
                for K in range(2):
                    for i in range(2):
                        nc.vector.tensor_copy(
                            xT8p[:, K, i, h * XH:(h + 1) * XH],
                            xT8[:, K, 2 * h * XH + i:2 * (h + 1) * XH:2],
                        )

            def x_sq_act(h):
                for t in range(h * 8, (h + 1) * 8):
                    xsq = sp.tile([P, D], f32, tag="xsq")
                    nc.scalar.activation(
                        xsq[:], xst[:, t, :], AF.Square,
                        accum_out=x2_all[:, t:t + 1],
                    )

            def x_sq_dve(h):
                for t in range(h * 8, (h + 1) * 8):
                    xsq = sp.tile([P, D], fp8, tag="xsq8")
                    nc.vector.scalar_tensor_tensor(
                        xsq[:], xst[:, t, :], 1.0, xst[:, t, :],
                        ALU.mult, ALU.mult, accum_out=x2_all[:, t:t + 1],
                    )

            def x2sc_piece(h, eng):
                eng.tensor_scalar(
                    x2sc[:, h * 8:(h + 1) * 8],
                    x2_all[:, h * 8:(h + 1) * 8],
                    negscale[:], 0.0, ALU.mult, ALU.add,
                )

            sv_stage = {}

            def sv_load(ch, half=None, eng=None):
                rows = RCH if half is None else RCH // 2
                r0 = ch * RCH + (0 if not half else RCH // 2)
                svst = stp.tile([P, rows // P, D], f32, tag=f"svst{rows}",
                                bufs=(2 if half is not None else 1))
                sv_stage[(ch, half)] = svst
                (eng or nc.sync).dma_start(
                    svst[:],
                    Sd[r0:r0 + rows, :].rearrange("(j p) d -> p j d", p=P),
                )

            def sv_cast_store(ch, half=None):
                svst = sv_stage.pop((ch, half))
                rows = RCH if half is None else RCH // 2
                r0 = ch * RCH + (0 if not half else RCH // 2)
                sv8 = s8p.tile([P, rows // P, D], fp8, tag=f"sv8{rows}",
                               bufs=(2 if half is not None else 1))
                nc.vector.tensor_scalar_mul(sv8[:], svst[:], 2.0)
                nc.scalar.dma_start(
                    s8d[r0:r0 + rows, :].rearrange("(j p) d -> p j d", p=P),
                    sv8[:],
                )

            def sv_cast_store2(ch):
                # halved cast/store/transpose off one staged load (SP queue)
                svst = sv_stage.pop((ch, None))
                H = JCH // 2
                for hh in range(2):
                    r0 = ch * RCH + hh * (RCH // 2)
                    sv8 = s8p.tile([P, H, D], fp8, tag="sv8h2")
                    nc.vector.tensor_scalar_mul(
                        sv8[:], svst[:, hh * H:(hh + 1) * H, :], 2.0)
                    nc.sync.dma_start(
                        s8d[r0:r0 + RCH // 2, :].rearrange(
                            "(j p) d -> p j d", p=P),
                        sv8[:],
                    )
                    nc.sync.dma_start_transpose(
                        svsT8[ch].bitcast(u16)[
                            :, :, hh * RCH // 2:(hh + 1) * RCH // 2],
                        s8d.bitcast(u16)[r0:r0 + RCH // 2, :],
                    )
                    for q in (0, 1):
                        lo = 2 * (hh * RCH // 2 + q * NCH)
                        hi = lo + 2 * NCH
                        eng = nc.gpsimd if q % 2 == 0 else nc.vector
                        eng.tensor_tensor(
                            sqT8[ch][:, :, lo:hi],
                            svsT8[ch][:, :, lo:hi],
                            svsT8[ch][:, :, lo:hi],
                            ALU.mult,
                        )

            def sv_transp(ch, half=None):
                rows = RCH if half is None else RCH // 2
                r0 = ch * RCH + (0 if not half else RCH // 2)
                c0 = r0 - ch * RCH
                nc.scalar.dma_start_transpose(
                    svsT8[ch].bitcast(u16)[:, :, c0:c0 + rows],
                    s8d.bitcast(u16)[r0:r0 + rows, :],
                )

            def sv_square(ch):
                # split the 4 512-n slices between Pool and DVE
                for q in range(RCH // NCH):
                    lo = 2 * (q * NCH)
                    hi = lo + 2 * NCH
                    eng = nc.gpsimd if q == 0 else nc.vector
                    eng.tensor_tensor(
                        sqT8[ch][:, :, lo:hi],
                        svsT8[ch][:, :, lo:hi],
                        svsT8[ch][:, :, lo:hi],
                        ALU.mult,
                    )

            # ---------- emission ----------
            x_load(0)
            x_sq_act(0)
            x_cast(0)
            sv_load(0, 0)
            sv_load(0, 1)
            sv_cast_store(0, 0)
            sv_transp(0, 0)
            sv_cast_store(0, 1)
            sv_transp(0, 1)
            sv_square(0)
            x_store_transp(0)
            x2sc_piece(0, nc.vector)
            with tc.tile_wait_until(0.024):
                x_load(1)
            with tc.tile_wait_until(0.029):
                sv_load(1)
            x_cast(1)
            x_store_transp(1)
            sv_cast_store(1)
            sv_transp(1)
            sv_square(1)
            x_sq_dve(1)
            x2sc_piece(1, nc.vector)

            # ---------- main loop ----------
            def mm_group(g):
                for t in range(MT):
                    pw = pp.tile([P, NG], f32, tag="pw")
                    for c in range(NG // NCH):
                        n0 = c * NCH
                        bank = pw[:, c * NCH:(c + 1) * NCH]
                        for K in range(2):
                            nc.tensor.matmul(
                                bank,
                                xT8p[:, K, :, t * P:(t + 1) * P],
                                drpair(svsT8[g][:, K, :], n0, NCH),
                                start=(K == 0),
                                stop=False,
                                perf_mode=DR,
                            )
                        for K in range(2):
                            nc.tensor.matmul(
                                bank,
                                neg_q[:],
                                drpair(sqT8[g][:, K, :], n0, NCH),
                                start=False,
                                stop=(K == 1),
                                perf_mode=DR,
                            )
                    nc.scalar.activation(
                        pw[:], pw[:], AF.Exp,
                        bias=x2sc[:, t:t + 1], scale=scale_bc[:],
                        accum_out=partials[:, t, g:g + 1],
                    )

            mm_group(0)
            nc.sync.dma_start(
                outd[0].rearrange("(t p) -> p t", p=P), partials[:, :, 0]
            )
            mm_group(1)
            nc.sync.dma_start(
                outd[1].rearrange("(t p) -> p t", p=P), partials[:, :, 1]
            )

    nc.finalize()
    return nc


def kernel(X: np.ndarray, svs: np.ndarray, scale: np.ndarray) -> np.ndarray:
    from concourse.bass_utils import run_bass_kernel_spmd

    if "nc" not in _CACHE:
        _CACHE["nc"] = _build_nc()
    nc = _CACHE["nc"]

    X = np.ascontiguousarray(X, dtype=np.float32)
    svs = np.ascontiguousarray(svs, dtype=np.float32)
    sc = np.asarray(scale, dtype=np.float32).reshape(1)

    in_maps = [
        {
            "X": X[(i % XSH) * M_LOC:(i % XSH + 1) * M_LOC],
            "svs": svs[(i // XSH) * N_LOC:(i // XSH + 1) * N_LOC],
            "scale": sc,
        }
        for i in range(NCORES)
    ]
    res = run_bass_kernel_spmd(nc, in_maps, core_ids=list(range(NCORES)))
    T = [r["out"].reshape(NGRP, M_LOC).astype(np.float64).sum(axis=0)
         for r in res.results]
    C = float(-np.log(N_FULL) + (D / 2) * np.log(float(sc[0]) / np.pi))
    out = np.concatenate(
        [np.log(T[q] + T[q + XSH]) + C for q in range(XSH)]
    )
    return out.astype(np.float32)
